# revision 35
# baseline (speedup 1.0000x reference)
"""Multi-head causal attention (B=2, S=2048, D=1024, H=16) on 8 NeuronCores.

Sharding v4: 2-way data parallel over batch x 4-way tensor parallel over
heads (core c handles batch c//4, heads 4*(c%4)..4*(c%4)+3). Each core
computes q/k/v projections for its 4 heads over its batch's 2048 tokens,
causal attention, and a partial output projection (its 256 rows of
W_proj); the host sums 4 partials per batch and adds b_proj.

Device-side design (all matmuls bf16 with fp32 PSUM accumulate):
 - x arrives pre-transposed and tiled [128, 4 col-tiles, 8 ks, 512] so
   every input DMA chunk is >=1KB-contiguous per partition.
 - q and k are produced transposed in 2-head tiles ([128, 2048]: head
   pair hp with even head dims in partitions 0:64, odd in 64:128 --
   exactly how the projection psum lands, so one bias-add each).
 - scores are computed as ST = K @ Q^T ([keys, queries]) with TWO
   K=64 matmuls ROW-PACKED into one PE pass via tile_position row
   bands (rows 0:64 = even head, 64:128 = odd head), so a head-pair's
   scores cost one matmul's wall time. One Exp per kj tile covers both
   heads ([128, 2, 512] psum -> bf16); causal-triangle masking of diag
   tiles runs on the otherwise-idle GpSimd.
 - v is produced token-major [tokens, 260] = [V_h|1]x4; the AV product
   expST.T @ [V|1] yields context AND the softmax denominator in one
   accumulation group with queries on PSUM partitions.
 - the schedule zippers 2-kj score chunks between ~2us filler units of
   PE work (projections, per-r AV chains, lagged output groups) so the
   in-order PE stream never sits behind a score matmul waiting for the
   Exp stream to free a qk psum buffer; a few round-3 score chunks are
   pulled into round 2's scalar slack to shorten the exp-bound tail.
 - input DMA uses only TWO issuing queues (sync+gpsimd, ~16 software
   DGE procs each): every proc's final clock value is waited out
   serially by every engine in the epilogue, so fewer queues shorten
   both the head sem-init and the tail drain.
 - output is stored partition-major ([128, 16*1024] per core) so out
   DMAs have 4KB-contiguous descriptors (the kernel tail is limited by
   the queues' per-descriptor feed rate, not bytes); the host undoes
   the tiling. The last group runs a fused per-tile drain: AV chain ->
   norm -> transpose -> out-proj, with the out-proj lagged one tile
   behind its transpose and pair-batched 1MB output DMAs.
"""

import sys

sys.path.insert(0, "/opt/trn_rl_repo")

import numpy as np
import ml_dtypes

import concourse.bass as bass
import concourse.mybir as mybir
import concourse.tile as tile
from concourse import bacc
from concourse.bass_utils import run_bass_kernel_spmd

BF16 = mybir.dt.bfloat16
F32 = mybir.dt.float32
NPBF16 = ml_dtypes.bfloat16

B, S, D = 2, 2048, 1024
H, DH = 16, 64
HC = 4               # heads per core
T = S                # tokens per core (one batch)
KS = D // 128        # 8 contraction subtiles
QT = T // 128        # 16 query tiles
NCOL = 4             # 512-token projection column tiles
ACT_F = mybir.ActivationFunctionType


def _build_nc():
    # Bacc (not raw Bass): its compile() pass pipeline splits multi-sem
    # waits down to the TRN2 1-wait-per-instruction hardware limit.
    nc = bacc.Bacc("TRN2", target_bir_lowering=False, debug=False, num_devices=8)

    xT = nc.dram_tensor("xT", [128, NCOL, KS, 512], BF16, kind="ExternalInput")
    wq = nc.dram_tensor("wq", [128, 2, KS, 128], BF16, kind="ExternalInput")
    wk = nc.dram_tensor("wk", [128, 2, KS, 128], BF16, kind="ExternalInput")
    wv = nc.dram_tensor("wv", [128, KS, 260], BF16, kind="ExternalInput")
    bq = nc.dram_tensor("bq", [128, 2], F32, kind="ExternalInput")
    bk = nc.dram_tensor("bk", [128, 2], F32, kind="ExternalInput")
    bv = nc.dram_tensor("bv", [1, 260], BF16, kind="ExternalInput")
    wp = nc.dram_tensor("wp", [128, 2, D], BF16, kind="ExternalInput")
    tri = nc.dram_tensor("tri", [128, 128], BF16, kind="ExternalInput")
    # partition-major output: o[p, tt*1024 + d] = out[tt*128 + p, d]
    out = nc.dram_tensor("o", [128, QT * D], BF16, kind="ExternalOutput")

    with tile.TileContext(nc) as tc:
        with (
            tc.tile_pool(name="singles", bufs=1) as singles,
            # one psum pool: tag "qk" [128,2,512] f32 = 2 banks x 2 bufs,
            # tag "av" [128,4,66] = 1 bank x 2, tag "po" [128,512] = 1 bank
            # x 2 -> exactly 8 banks
            tc.tile_pool(name="qkps", bufs=2, space="PSUM") as qkps,
            tc.tile_pool(name="expp", bufs=44) as expp,
            tc.tile_pool(name="ctxp", bufs=6) as ctxp,
            tc.tile_pool(name="outp", bufs=2) as outp,
            tc.tile_pool(name="rdp", bufs=4) as rdp,
        ):
            # ---- resident tensors -------------------------------------
            wq_sb = singles.tile([128, 2, KS, 128], BF16, tag="wq")
            wk_sb = singles.tile([128, 2, KS, 128], BF16, tag="wk")
            wv_sb = singles.tile([128, KS, 260], BF16, tag="wv")
            bq_sb = singles.tile([128, 2], F32, tag="bq")
            bk_sb = singles.tile([128, 2], F32, tag="bk")
            # b_v (+ the ones columns) broadcast to all partitions; fused
            # into the v copyback as a tensor_tensor add on DVE
            bv_sb = singles.tile([128, 260], BF16, tag="bv")
            wp_sb = singles.tile([128, 2, D], BF16, tag="wp")
            tri_sb = singles.tile([128, 128], BF16, tag="tri")
            xT_sb = singles.tile([128, NCOL, KS, 512], BF16, tag="xT")
            # q for heads (0,1) in qT[0] rows (0:64|64:128), (2,3) in qT[1]
            qT = [
                singles.tile([128, T], BF16, tag=f"qT{i}", name=f"qT{i}")
                for i in range(2)
            ]
            # k likewise: head pair hp, even head in rows 0:64, odd 64:128
            kT2 = [
                singles.tile([128, T], BF16, tag=f"kT{i}", name=f"kT{i}")
                for i in range(2)
            ]
            # v, per key-tile: [V_h0 | 1 | V_h1 | 1 | V_h2 | 1 | V_h3 | 1]
            v_sb = singles.tile([128, QT, 260], BF16, tag="v")
            # ctxT: dims of heads (0,1) in [0], (2,3) in [1]; matches wp rows
            ctxT = [
                singles.tile([128, QT, 128], BF16, tag=f"ctxT{i}", name=f"ctxT{i}")
                for i in range(2)
            ]

            # ---- input DMA: only TWO issuing queues (sync + gpsimd).
            # Each DMA-issuing engine-queue costs ~16 software-DGE procs
            # whose final clock values every engine must wait out in the
            # kernel epilogue (~115ns per proc per engine), so fewer
            # queues = shorter head sem-init AND tail drain. Two queues
            # still saturate HBM (~183GB/s each). Need-order: the q
            # chain's operands lead both queues; wk follows 256KB behind
            # so the k chain is fed by its ~15us start.
            nc.gpsimd.dma_start(wq_sb[:, 0, 0:2, :], wq[:, 0, 0:2, :])
            nc.sync.dma_start(xT_sb[:, 0, 0:2, :], xT[:, 0, 0:2, :])
            nc.gpsimd.dma_start(wq_sb[:, 0, 2:8, :], wq[:, 0, 2:8, :])
            nc.sync.dma_start(xT_sb[:, 0, 2:4, :], xT[:, 0, 2:4, :])
            nc.gpsimd.dma_start(xT_sb[:, 0, 4:8, :], xT[:, 0, 4:8, :])
            nc.sync.dma_start(wk_sb[:, 0, :, :], wk[:, 0, :, :])
            nc.sync.dma_start(bq_sb[:], bq[:])
            nc.sync.dma_start(bk_sb[:], bk[:])
            nc.sync.dma_start(bv_sb[:], bv[:].to_broadcast((128, 260)))
            nc.sync.dma_start(tri_sb[:], tri[:])
            nc.gpsimd.dma_start(wq_sb[:, 1, :, :], wq[:, 1, :, :])
            nc.gpsimd.dma_start(wk_sb[:, 1, :, :], wk[:, 1, :, :])
            nc.sync.dma_start(xT_sb[:, 1, 0:4, :], xT[:, 1, 0:4, :])
            nc.gpsimd.dma_start(xT_sb[:, 1, 4:8, :], xT[:, 1, 4:8, :])
            nc.sync.dma_start(wv_sb[:], wv[:])
            nc.sync.dma_start(xT_sb[:, 2, :, :], xT[:, 2, :, :])
            nc.sync.dma_start(wp_sb[:], wp[:])
            nc.sync.dma_start(xT_sb[:, 3, :, :], xT[:, 3, :, :])

            # ---- phase emitters ---------------------------------------
            def emit_proj_qk(tcol, halves=(0, 1), which="qk"):
                """q and/or k projection chains for one 512-token column
                tile (each chain is a ~2us filler unit)."""
                csl = bass.ds(tcol * 512, 512)
                for half in halves:  # heads (0,1) then (2,3)
                    if "q" in which:
                        ps_q = qkps.tile(
                            [128, 512], F32, tag="po", name="ps_q", bufs=2
                        )
                        for ks in range(KS):
                            nc.tensor.matmul(
                                ps_q[:],
                                wq_sb[:, half, ks, :],
                                xT_sb[:, tcol, ks, :],
                                start=(ks == 0),
                                stop=(ks == KS - 1),
                            )
                        nc.vector.tensor_scalar_add(
                            qT[half][:, csl], ps_q[:], bq_sb[:, half : half + 1]
                        )
                    if "k" in which:
                        ps_k = qkps.tile(
                            [128, 512], F32, tag="po", name="ps_k", bufs=2
                        )
                        for ks in range(KS):
                            nc.tensor.matmul(
                                ps_k[:],
                                wk_sb[:, half, ks, :],
                                xT_sb[:, tcol, ks, :],
                                start=(ks == 0),
                                stop=(ks == KS - 1),
                            )
                        nc.vector.tensor_scalar_add(
                            kT2[half][:, csl], ps_k[:], bk_sb[:, half : half + 1]
                        )

            def emit_proj_v(tcol, jjs=(0, 1, 2, 3)):
                """v projections, one 128-token tile each."""
                for jj in jjs:
                    tt = tcol * 4 + jj
                    ps_v = qkps.tile([128, 512], F32, tag="po", name="ps_v", bufs=2)
                    for ks in range(KS):
                        nc.tensor.matmul(
                            ps_v[:, :260],
                            xT_sb[:, tcol, ks, bass.ds(jj * 128, 128)],
                            wv_sb[:, ks, :],
                            start=(ks == 0),
                            stop=(ks == KS - 1),
                        )
                    # bias add also writes the ones columns (65h+64);
                    # DVE not GpSimd: GPSIMD cannot read PSUM
                    nc.vector.tensor_add(v_sb[:, tt, :], ps_v[:, :260], bv_sb[:])

            ex_tiles = {}  # (hp, g) -> list of per-kj exp tiles [128,2,512]

            def emit_scores(hp, g, kjs):
                """Row-packed scores + exp + diag masks for head pair hp,
                query group g, key tiles kjs. Even head streams through PE
                rows 0:64, odd head rows 64:128, concurrently."""
                exl = ex_tiles.setdefault((hp, g), [])
                for kj in kjs:
                    assert kj == len(exl)
                    qk = qkps.tile([128, 2, 512], F32, tag="qk", name=f"qk_p{hp}")
                    # queries strictly below kj contribute nothing
                    ri = max(0, kj - 4 * g)
                    cs = bass.ds(ri * 128, 512 - ri * 128)
                    qsl = bass.ds(g * 512 + ri * 128, 512 - ri * 128)
                    ksl = bass.ds(kj * 128, 128)
                    for par in range(2):
                        psl = bass.ds(64 * par, 64)
                        nc.tensor.matmul(
                            qk[:, par, cs],
                            kT2[hp][psl, ksl],
                            qT[hp][psl, qsl],
                            start=True,
                            stop=True,
                        )
                    ex = expp.tile([128, 2, 512], BF16, tag="exp")
                    nc.scalar.activation(
                        ex[:, :, cs], qk[:, :, cs], ACT_F.Exp, scale=0.125
                    )
                    exl.append(ex)
                    if kj >= 4 * g:  # diagonal block: zero the masked triangle
                        r = kj - 4 * g
                        dsl = bass.ds(r * 128, 128)
                        for par in range(2):
                            nc.gpsimd.tensor_mul(
                                ex[:, par, dsl], ex[:, par, dsl], tri_sb[:]
                            )

            _ctx_cache = {}

            def _ctx_for(g):
                if g not in _ctx_cache:
                    _ctx_cache[g] = ctxp.tile(
                        [128, 4, 256], BF16, tag="ctx", name=f"ctx{g}"
                    )
                return _ctx_cache[g]

            def emit_av(h, g, rs=(0, 1, 2, 3)):
                """AV + normalize for head h, group g, query tiles rs
                (contiguous); fills ctx (+ctxT via DMA transpose for odd
                heads, whose pair rows are then complete). Each call
                allocates its OWN psum tile: a tile cached across calls
                would still receive writes after later ring allocations,
                which breaks the pool's lifetime tracking (observed as a
                nondeterministic accuracy race)."""
                exl = ex_tiles[(h // 2, g)]
                ctx_t = _ctx_for(g)
                hsl = bass.ds(64 * h, 64)
                av = qkps.tile([128, 4, 66], F32, tag="av", name=f"av{h}", bufs=2)
                for r in rs:
                    qi = 4 * g + r
                    for kj in range(qi + 1):
                        nc.tensor.matmul(
                            av[:, r, 0:65],
                            exl[kj][:, h % 2, bass.ds(r * 128, 128)],
                            v_sb[:, kj, bass.ds(65 * h, 65)],
                            start=(kj == 0),
                            stop=(kj == qi),
                        )
                r0, nr = rs[0], len(rs)
                rd = rdp.tile([128, 4], F32, tag="rd")
                nc.vector.reciprocal(rd[:, 0:nr], av[:, r0 : r0 + nr, 64:65])
                for i, r in enumerate(rs):
                    nc.vector.tensor_scalar_mul(
                        ctx_t[:, r, hsl], av[:, r, 0:64], rd[:, i : i + 1]
                    )
                if h % 2 == 1:  # heads (h-1, h) pair complete -> transpose
                    half = h // 2
                    for r in rs:
                        tt = 4 * g + r
                        nc.sync.dma_start(
                            ctxT[half][:, tt, :],
                            ctx_t[:, r, bass.ds(128 * half, 128)],
                            transpose=True,
                        )

            def emit_out_pair(g, p):
                """Output projection + 512KB DMA for tiles (2p, 2p+1) of
                group g."""
                og = outp.tile([128, 2, D], BF16, tag="og", name="og")
                for rr in range(2):
                    r = 2 * p + rr
                    tt = g * 4 + r
                    for half in range(2):
                        po = qkps.tile([128, 512], F32, tag="po", name="ps_o", bufs=2)
                        for i in range(2):
                            nc.tensor.matmul(
                                po[:],
                                ctxT[i][:, tt, :],
                                wp_sb[:, i, bass.ds(half * 512, 512)],
                                start=(i == 0),
                                stop=(i == 1),
                            )
                        nc.vector.tensor_copy(
                            og[:, rr, bass.ds(half * 512, 512)], po[:]
                        )
                eng = (nc.gpsimd, nc.sync)[(2 * g + p) % 2]
                eng.dma_start(
                    out[:, bass.ds((g * 4 + 2 * p) * D, 2 * D)], og[:]
                )
                if p == 1:
                    _ctx_cache.pop(g, None)

            def emit_drain_av(g, r):
                """Drain stage 1 for the LAST group, head 3, query tile r:
                AV chain -> norm -> transpose. Emittable once pair-1
                exps cover kj <= 4g+r; the lagged dr_po pipeline hides
                the transpose latency for all but the last tile."""
                h = HC - 1
                exl = ex_tiles[(h // 2, g)]
                ctx_t = _ctx_for(g)
                hsl = bass.ds(64 * h, 64)
                qi = 4 * g + r
                av = qkps.tile([128, 4, 66], F32, tag="av", name="av_dr", bufs=2)
                for kj in range(qi + 1):
                    nc.tensor.matmul(
                        av[:, 0, 0:65],
                        exl[kj][:, h % 2, bass.ds(r * 128, 128)],
                        v_sb[:, kj, bass.ds(65 * h, 65)],
                        start=(kj == 0),
                        stop=(kj == qi),
                    )
                rd = rdp.tile([128, 4], F32, tag="rd", name="rd_dr")
                nc.vector.reciprocal(rd[:, 0:1], av[:, 0, 64:65])
                nc.vector.tensor_scalar_mul(
                    ctx_t[:, r, hsl], av[:, 0, 0:64], rd[:, 0:1]
                )
                nc.sync.dma_start(
                    ctxT[1][:, 4 * g + r, :],
                    ctx_t[:, r, bass.ds(128, 128)],
                    transpose=True,
                )

            _dr_od = {}

            def emit_drain_po(g, r):
                """Drain stage 2: out-proj for tile r, lagged one tile
                behind its transpose. Output accumulates into a 2-tile
                [128,2,D] buffer DMA'd once per PAIR with 4KB-contiguous
                descriptors: the kernel tail is limited by the DMA
                queues' ~23ns/descriptor feed rate, not bytes, so
                halving descriptor count directly shortens the drain."""
                tt = 4 * g + r
                if r % 2 == 0:
                    # "og" tag ring: no other og allocation is emitted
                    # between this one and its pair-completing DMA below
                    _dr_od["t"] = outp.tile([128, 2, D], BF16, tag="og", name="od")
                od = _dr_od["t"]
                for half in range(2):
                    po = qkps.tile([128, 512], F32, tag="po", name="ps_o", bufs=2)
                    for i in range(2):
                        nc.tensor.matmul(
                            po[:],
                            ctxT[i][:, tt, :],
                            wp_sb[:, i, bass.ds(half * 512, 512)],
                            start=(i == 0),
                            stop=(i == 1),
                        )
                    osl = bass.ds(half * 512, 512)
                    # split drain casts: ScalarE is free after exps
                    if half == 1:
                        nc.scalar.copy(od[:, r % 2, osl], po[:])
                    else:
                        nc.vector.tensor_copy(od[:, r % 2, osl], po[:])
                if r == 1:
                    nc.gpsimd.dma_start(
                        out[:, bass.ds((tt - 1) * D, 2 * D)], od[:]
                    )
                elif r == 3:
                    # the very last transfer: split by partition rows
                    # across BOTH queues so each side feeds only 64
                    # descriptors (~1.5us) after the final casts
                    osl = bass.ds((tt - 1) * D, 2 * D)
                    nc.gpsimd.dma_start(out[0:64, osl], od[0:64, :, :])
                    nc.sync.dma_start(out[64:128, osl], od[64:128, :, :])
                    _ctx_cache.pop(g, None)

            # ---- schedule ---------------------------------------------
            # Zipper: 2-kj score chunks (0.45us PE feeding 1.9us of Exp)
            # alternate with ~2us filler units so the Exp stream runs
            # continuously without the in-order PE stream ever blocking
            # on a qk psum buffer. AV r-chains are emitted only after the
            # score chunks covering their kj range (PE is in-order: an AV
            # matmul emitted before its exp's score matmul would deadlock).
            s, pq, pv = emit_scores, emit_proj_qk, emit_proj_v
            av, op = emit_av, emit_out_pair
            dr_av, dr_po = emit_drain_av, emit_drain_po
            # round 0 (PE-rich: projections dominate; exp stream has slack)
            pq(0, halves=(0,))
            s(0, 0, (0, 1))
            pq(0, halves=(1,), which="q")
            s(0, 0, (2, 3))
            pq(0, halves=(1,), which="k")
            s(1, 0, (0, 1))
            pv(0, (0, 1))
            s(1, 0, (2, 3))
            pv(0, (2, 3))
            av(0, 0); av(1, 0)
            pq(1, halves=(0,))
            av(2, 0); av(3, 0)
            pq(1, halves=(1,))
            # round 1
            s(0, 1, (0, 1)); pv(1, (0, 1))
            s(0, 1, (2, 3)); pv(1, (2, 3))
            s(0, 1, (4, 5)); pq(2, halves=(0,), which="q")
            s(0, 1, (6, 7)); pq(2, halves=(0,), which="k")
            s(1, 1, (0, 1)); pq(2, halves=(1,), which="q")
            s(1, 1, (2, 3)); pq(2, halves=(1,), which="k")
            s(1, 1, (4, 5)); av(0, 1, (0, 1)); op(0, 0)
            s(1, 1, (6, 7)); av(0, 1, (2, 3)); av(1, 1, (0, 1))
            av(1, 1, (2, 3)); op(0, 1); av(2, 1); av(3, 1)
            # round 2 (scalar tightens: ~1.5us filler per 2-kj chunk)
            s(0, 2, (0, 1)); pv(2, (0, 1))
            s(0, 2, (2, 3)); pv(2, (2, 3))
            s(0, 2, (4, 5)); pq(3, halves=(0,), which="q")
            s(0, 2, (6, 7)); pq(3, halves=(0,), which="k")
            s(0, 2, (8, 9)); pq(3, halves=(1,), which="q")
            s(0, 2, (10, 11)); pq(3, halves=(1,), which="k")
            s(1, 2, (0, 1)); op(1, 0)
            s(1, 2, (2, 3)); s(0, 3, (0, 1)); op(1, 1)
            s(1, 2, (4, 5)); s(0, 3, (2, 3)); av(0, 2, (0, 1))
            s(1, 2, (6, 7)); av(0, 2, (2, 3))
            s(1, 2, (8, 9)); av(1, 2, (0, 1))
            s(1, 2, (10, 11)); av(1, 2, (2, 3))
            # round 3: round-2 AV leftovers lead so their ctxT transposes
            # get ~6us of lead before op(2,*) consumes them; the first 4
            # pair-0 kjs were pulled into round 2's scalar slack above
            s(0, 3, (4, 5)); av(2, 2, (0, 1))
            s(0, 3, (6, 7)); av(2, 2, (2, 3))
            s(0, 3, (8, 9)); av(3, 2, (0, 1))
            s(0, 3, (10, 11)); av(3, 2, (2, 3))
            s(0, 3, (12,)); pv(3, (0, 1))
            s(0, 3, (13,)); pv(3, (2, 3))
            s(0, 3, (14,)); op(2, 0)
            s(0, 3, (15,)); av(0, 3, (0, 1))
            s(1, 3, (0, 1)); av(0, 3, (2, 3))
            s(1, 3, (2, 3)); av(1, 3, (0, 1))
            s(1, 3, (4, 5)); av(1, 3, (2, 3))
            s(1, 3, (6, 7)); op(2, 1)
            s(1, 3, (8, 9))
            s(1, 3, (10, 11))
            s(1, 3, (12,)); av(2, 3, (0,))
            s(1, 3, (13,)); av(2, 3, (1,)); dr_av(3, 0)
            s(1, 3, (14,)); av(2, 3, (2,)); dr_av(3, 1); dr_po(3, 0)
            s(1, 3, (15,)); dr_av(3, 2); av(2, 3, (3,)); dr_po(3, 1)
            dr_av(3, 3); dr_po(3, 2); dr_po(3, 3)
            # av-tag psum ring (2 slots) allocation order in this tail:
            # av23r0 A, av23r1 B, drav0 A, av23r2 B, drav1 A, drav2 B,
            # av23r3 A, drav3 B -- every allocation's slot predecessor is
            # fully emitted and consumed by then.

    return nc


_NC_CACHE = None


def _get_nc():
    global _NC_CACHE
    if _NC_CACHE is None:
        nc = _build_nc()
        nc.finalize()  # runs Bacc's pass pipeline (sync-wait splitting etc.)
        _NC_CACHE = nc
    return _NC_CACHE


def _make_in_maps(x, W_qkv, b_qkv, W_proj):
    tri = np.triu(np.ones((128, 128), dtype=np.float32)).astype(NPBF16)

    def wtile(w):  # [D, M] -> [128, KS, M] contraction-major tiles
        m = w.shape[1]
        return np.ascontiguousarray(
            w.astype(NPBF16).reshape(KS, 128, m).transpose(1, 0, 2)
        )

    def wtile2(w):  # [D, 256] -> [128, 2 halves, KS, 128] half-major
        return np.ascontiguousarray(
            w.astype(NPBF16).reshape(KS, 128, 2, 128).transpose(1, 2, 0, 3)
        )

    # xT per batch: [S, D] -> [128, NCOL, KS, 512]
    xTs = [
        np.ascontiguousarray(
            x[b]
            .astype(NPBF16)
            .reshape(NCOL, 512, KS, 128)
            .transpose(3, 0, 2, 1)
        )
        for b in range(B)
    ]

    in_maps = []
    for c in range(8):
        b = c // 4
        hs = [4 * (c % 4) + i for i in range(HC)]
        cs = np.concatenate([np.arange(64 * h, 64 * h + 64) for h in hs])
        wq_c = W_qkv[:, 0 * D :][:, cs]                      # [D, 256]
        wk_c = W_qkv[:, 1 * D :][:, cs]
        v_blk = W_qkv[:, 2 * D :][:, cs].astype(np.float32)  # [D, 256]
        wv_c = np.zeros((D, 260), dtype=np.float32)
        bv_c = np.zeros((1, 260), dtype=np.float32)
        for i in range(HC):
            wv_c[:, 65 * i : 65 * i + 64] = v_blk[:, 64 * i : 64 * i + 64]
            bv_c[0, 65 * i : 65 * i + 64] = b_qkv[2 * D :][cs][64 * i : 64 * i + 64]
            bv_c[0, 65 * i + 64] = 1.0
        in_maps.append(
            {
                "xT": xTs[b],
                "wq": wtile2(wq_c),
                "wk": wtile2(wk_c),
                "wv": wtile(wv_c),
                "bq": np.ascontiguousarray(
                    b_qkv[0 * D :][cs].astype(np.float32).reshape(2, 128).T
                ),
                "bk": np.ascontiguousarray(
                    b_qkv[1 * D :][cs].astype(np.float32).reshape(2, 128).T
                ),
                "bv": bv_c.astype(NPBF16),
                "wp": np.ascontiguousarray(
                    W_proj[cs, :].astype(NPBF16).reshape(2, 128, D).transpose(1, 0, 2)
                ),
                "tri": tri,
            }
        )
    return in_maps


def kernel(x, W_qkv, b_qkv, W_proj, b_proj, **run_kwargs):
    x = np.asarray(x, dtype=np.float32)
    W_qkv = np.asarray(W_qkv, dtype=np.float32)
    b_qkv = np.asarray(b_qkv, dtype=np.float32)
    W_proj = np.asarray(W_proj, dtype=np.float32)
    b_proj = np.asarray(b_proj, dtype=np.float32)

    nc = _get_nc()
    in_maps = _make_in_maps(x, W_qkv, b_qkv, W_proj)
    res = run_bass_kernel_spmd(nc, in_maps, core_ids=list(range(8)), **run_kwargs)

    out = np.zeros((B, S, D), dtype=np.float32)
    for c in range(8):
        b = c // 4
        # o[p, tt*D + d] = out[tt*128 + p, d]
        o = res.results[c]["o"].astype(np.float32).reshape(128, QT, D)
        out[b] += o.transpose(1, 0, 2).reshape(S, D)
    out += b_proj[None, None, :]
    kernel.last_result = res
    return out


# revision 36
# speedup vs baseline: 1.0857x; 1.0857x over previous
"""Multi-head causal attention (B=2, S=2048, D=1024, H=16) on 8 NeuronCores.

Sharding v4: 2-way data parallel over batch x 4-way tensor parallel over
heads (core c handles batch c//4, heads 4*(c%4)..4*(c%4)+3). Each core
computes q/k/v projections for its 4 heads over its batch's 2048 tokens,
causal attention, and a partial output projection (its 256 rows of
W_proj); the host sums 4 partials per batch and adds b_proj.

Device-side design (all matmuls bf16 with fp32 PSUM accumulate):
 - x arrives pre-transposed and tiled [128, 4 col-tiles, 8 ks, 512] so
   every input DMA chunk is >=1KB-contiguous per partition.
 - q and k are produced transposed in 2-head tiles ([128, 2048]: head
   pair hp with even head dims in partitions 0:64, odd in 64:128 --
   exactly how the projection psum lands, so one bias-add each).
 - scores are computed as ST = K @ Q^T ([keys, queries]) with TWO
   K=64 matmuls ROW-PACKED into one PE pass via tile_position row
   bands (rows 0:64 = even head, 64:128 = odd head), so a head-pair's
   scores cost one matmul's wall time. One Exp per kj tile covers both
   heads ([128, 2, 512] psum -> bf16); causal-triangle masking of diag
   tiles runs on the otherwise-idle GpSimd.
 - v is produced token-major [tokens, 260] = [V_h|1]x4; the AV product
   expST.T @ [V|1] yields context AND the softmax denominator in one
   accumulation group with queries on PSUM partitions.
 - the schedule zippers 2-kj score chunks between ~2us filler units of
   PE work (projections, per-r AV chains, lagged output groups) so the
   in-order PE stream never sits behind a score matmul waiting for the
   Exp stream to free a qk psum buffer; a few round-3 score chunks are
   pulled into round 2's scalar slack to shorten the exp-bound tail.
 - input DMA uses only TWO issuing queues (sync+gpsimd, ~16 software
   DGE procs each): every proc's final clock value is waited out
   serially by every engine in the epilogue, so fewer queues shorten
   both the head sem-init and the tail drain.
 - output is stored partition-major ([128, 16*1024] per core) so out
   DMAs have 4KB-contiguous descriptors (the kernel tail is limited by
   the queues' per-descriptor feed rate, not bytes); the host undoes
   the tiling. The last group runs a fused per-tile drain: AV chain ->
   norm -> transpose -> out-proj, with the out-proj lagged one tile
   behind its transpose and pair-batched 1MB output DMAs.
"""

import sys

sys.path.insert(0, "/opt/trn_rl_repo")

import numpy as np
import ml_dtypes

import concourse.bass as bass
import concourse.mybir as mybir
import concourse.tile as tile
from concourse import bacc
from concourse.bass_utils import run_bass_kernel_spmd

BF16 = mybir.dt.bfloat16
F32 = mybir.dt.float32
NPBF16 = ml_dtypes.bfloat16

B, S, D = 2, 2048, 1024
H, DH = 16, 64
HC = 4               # heads per core
T = S                # tokens per core (one batch)
KS = D // 128        # 8 contraction subtiles
QT = T // 128        # 16 query tiles
NCOL = 4             # 512-token projection column tiles
ACT_F = mybir.ActivationFunctionType


def _build_nc():
    # Bacc (not raw Bass): its compile() pass pipeline splits multi-sem
    # waits down to the TRN2 1-wait-per-instruction hardware limit.
    nc = bacc.Bacc("TRN2", target_bir_lowering=False, debug=False, num_devices=8)

    xT = nc.dram_tensor("xT", [128, NCOL, KS, 512], BF16, kind="ExternalInput")
    wq = nc.dram_tensor("wq", [128, 2, KS, 128], BF16, kind="ExternalInput")
    wk = nc.dram_tensor("wk", [128, 2, KS, 128], BF16, kind="ExternalInput")
    wv = nc.dram_tensor("wv", [128, KS, 260], BF16, kind="ExternalInput")
    bq = nc.dram_tensor("bq", [128, 2], F32, kind="ExternalInput")
    bk = nc.dram_tensor("bk", [128, 2], F32, kind="ExternalInput")
    bv = nc.dram_tensor("bv", [1, 260], BF16, kind="ExternalInput")
    wp = nc.dram_tensor("wp", [128, 2, D], BF16, kind="ExternalInput")
    tri = nc.dram_tensor("tri", [128, 128], BF16, kind="ExternalInput")
    # partition-major output: o[p, tt*1024 + d] = out[tt*128 + p, d]
    out = nc.dram_tensor("o", [128, QT * D], BF16, kind="ExternalOutput")

    with tile.TileContext(nc) as tc:
        with (
            tc.tile_pool(name="singles", bufs=1) as singles,
            # one psum pool: tag "qk" [128,2,512] f32 = 2 banks x 2 bufs,
            # tag "av" [128,4,66] = 1 bank x 2, tag "po" [128,512] = 1 bank
            # x 2 -> exactly 8 banks
            tc.tile_pool(name="qkps", bufs=2, space="PSUM") as qkps,
            tc.tile_pool(name="expp", bufs=44) as expp,
            tc.tile_pool(name="ctxp", bufs=6) as ctxp,
            tc.tile_pool(name="outp", bufs=2) as outp,
            tc.tile_pool(name="rdp", bufs=4) as rdp,
        ):
            # ---- resident tensors -------------------------------------
            wq_sb = singles.tile([128, 2, KS, 128], BF16, tag="wq")
            wk_sb = singles.tile([128, 2, KS, 128], BF16, tag="wk")
            wv_sb = singles.tile([128, KS, 260], BF16, tag="wv")
            bq_sb = singles.tile([128, 2], F32, tag="bq")
            bk_sb = singles.tile([128, 2], F32, tag="bk")
            # b_v (+ the ones columns) broadcast to all partitions; fused
            # into the v copyback as a tensor_tensor add on DVE
            bv_sb = singles.tile([128, 260], BF16, tag="bv")
            wp_sb = singles.tile([128, 2, D], BF16, tag="wp")
            tri_sb = singles.tile([128, 128], BF16, tag="tri")
            xT_sb = singles.tile([128, NCOL, KS, 512], BF16, tag="xT")
            # q for heads (0,1) in qT[0] rows (0:64|64:128), (2,3) in qT[1]
            qT = [
                singles.tile([128, T], BF16, tag=f"qT{i}", name=f"qT{i}")
                for i in range(2)
            ]
            # k likewise: head pair hp, even head in rows 0:64, odd 64:128
            kT2 = [
                singles.tile([128, T], BF16, tag=f"kT{i}", name=f"kT{i}")
                for i in range(2)
            ]
            # v, per key-tile: [V_h0 | 1 | V_h1 | 1 | V_h2 | 1 | V_h3 | 1]
            v_sb = singles.tile([128, QT, 260], BF16, tag="v")
            # ctxT: dims of heads (0,1) in [0], (2,3) in [1]; matches wp rows
            ctxT = [
                singles.tile([128, QT, 128], BF16, tag=f"ctxT{i}", name=f"ctxT{i}")
                for i in range(2)
            ]

            # ---- input DMA: only TWO issuing queues (sync + gpsimd).
            # Each DMA-issuing engine-queue costs ~16 software-DGE procs
            # whose final clock values every engine must wait out in the
            # kernel epilogue (~115ns per proc per engine), so fewer
            # queues = shorter head sem-init AND tail drain. Two queues
            # still saturate HBM (~183GB/s each). Need-order: the q
            # chain's operands lead both queues; wk follows 256KB behind
            # so the k chain is fed by its ~15us start.
            nc.gpsimd.dma_start(wq_sb[:, 0, 0:2, :], wq[:, 0, 0:2, :])
            nc.sync.dma_start(xT_sb[:, 0, 0:2, :], xT[:, 0, 0:2, :])
            nc.gpsimd.dma_start(wq_sb[:, 0, 2:8, :], wq[:, 0, 2:8, :])
            nc.sync.dma_start(xT_sb[:, 0, 2:4, :], xT[:, 0, 2:4, :])
            nc.gpsimd.dma_start(xT_sb[:, 0, 4:8, :], xT[:, 0, 4:8, :])
            nc.sync.dma_start(wk_sb[:, 0, :, :], wk[:, 0, :, :])
            nc.sync.dma_start(bq_sb[:], bq[:])
            nc.sync.dma_start(bk_sb[:], bk[:])
            nc.sync.dma_start(bv_sb[:], bv[:].to_broadcast((128, 260)))
            nc.sync.dma_start(tri_sb[:], tri[:])
            nc.gpsimd.dma_start(wq_sb[:, 1, :, :], wq[:, 1, :, :])
            nc.gpsimd.dma_start(wk_sb[:, 1, :, :], wk[:, 1, :, :])
            nc.sync.dma_start(xT_sb[:, 1, 0:4, :], xT[:, 1, 0:4, :])
            nc.gpsimd.dma_start(xT_sb[:, 1, 4:8, :], xT[:, 1, 4:8, :])
            nc.sync.dma_start(wv_sb[:], wv[:])
            nc.sync.dma_start(xT_sb[:, 2, :, :], xT[:, 2, :, :])
            nc.sync.dma_start(wp_sb[:], wp[:])
            nc.sync.dma_start(xT_sb[:, 3, :, :], xT[:, 3, :, :])

            # ---- phase emitters ---------------------------------------
            def emit_proj_qk(tcol, halves=(0, 1), which="qk"):
                """q and/or k projection chains for one 512-token column
                tile (each chain is a ~2us filler unit)."""
                csl = bass.ds(tcol * 512, 512)
                for half in halves:  # heads (0,1) then (2,3)
                    if "q" in which:
                        ps_q = qkps.tile(
                            [128, 512], F32, tag="po", name="ps_q", bufs=2
                        )
                        for ks in range(KS):
                            nc.tensor.matmul(
                                ps_q[:],
                                wq_sb[:, half, ks, :],
                                xT_sb[:, tcol, ks, :],
                                start=(ks == 0),
                                stop=(ks == KS - 1),
                            )
                        nc.vector.tensor_scalar_add(
                            qT[half][:, csl], ps_q[:], bq_sb[:, half : half + 1]
                        )
                    if "k" in which:
                        ps_k = qkps.tile(
                            [128, 512], F32, tag="po", name="ps_k", bufs=2
                        )
                        for ks in range(KS):
                            nc.tensor.matmul(
                                ps_k[:],
                                wk_sb[:, half, ks, :],
                                xT_sb[:, tcol, ks, :],
                                start=(ks == 0),
                                stop=(ks == KS - 1),
                            )
                        nc.vector.tensor_scalar_add(
                            kT2[half][:, csl], ps_k[:], bk_sb[:, half : half + 1]
                        )

            def emit_proj_v(tcol, jjs=(0, 1, 2, 3)):
                """v projections, one 128-token tile each."""
                for jj in jjs:
                    tt = tcol * 4 + jj
                    ps_v = qkps.tile([128, 512], F32, tag="po", name="ps_v", bufs=2)
                    for ks in range(KS):
                        nc.tensor.matmul(
                            ps_v[:, :260],
                            xT_sb[:, tcol, ks, bass.ds(jj * 128, 128)],
                            wv_sb[:, ks, :],
                            start=(ks == 0),
                            stop=(ks == KS - 1),
                        )
                    # bias add also writes the ones columns (65h+64);
                    # DVE not GpSimd: GPSIMD cannot read PSUM
                    nc.vector.tensor_add(v_sb[:, tt, :], ps_v[:, :260], bv_sb[:])

            ex_tiles = {}  # (hp, g) -> list of per-kj exp tiles [128,2,512]

            def emit_scores(hp, g, kjs):
                """Row-packed scores + exp + diag masks for head pair hp,
                query group g, key tiles kjs. Even head streams through PE
                rows 0:64, odd head rows 64:128, concurrently."""
                exl = ex_tiles.setdefault((hp, g), [])
                for kj in kjs:
                    assert kj == len(exl)
                    qk = qkps.tile([128, 2, 512], F32, tag="qk", name=f"qk_p{hp}")
                    # queries strictly below kj contribute nothing
                    ri = max(0, kj - 4 * g)
                    cs = bass.ds(ri * 128, 512 - ri * 128)
                    qsl = bass.ds(g * 512 + ri * 128, 512 - ri * 128)
                    ksl = bass.ds(kj * 128, 128)
                    for par in range(2):
                        psl = bass.ds(64 * par, 64)
                        nc.tensor.matmul(
                            qk[:, par, cs],
                            kT2[hp][psl, ksl],
                            qT[hp][psl, qsl],
                            start=True,
                            stop=True,
                        )
                    ex = expp.tile([128, 2, 512], BF16, tag="exp")
                    nc.scalar.activation(
                        ex[:, :, cs], qk[:, :, cs], ACT_F.Exp, scale=0.125
                    )
                    exl.append(ex)
                    if kj >= 4 * g:  # diagonal block: zero the masked triangle
                        r = kj - 4 * g
                        dsl = bass.ds(r * 128, 128)
                        for par in range(2):
                            nc.gpsimd.tensor_mul(
                                ex[:, par, dsl], ex[:, par, dsl], tri_sb[:]
                            )

            _ctx_cache = {}

            def _ctx_for(g):
                if g not in _ctx_cache:
                    _ctx_cache[g] = ctxp.tile(
                        [128, 4, 256], BF16, tag="ctx", name=f"ctx{g}"
                    )
                return _ctx_cache[g]

            def emit_av(h, g, rs=(0, 1, 2, 3)):
                """AV + normalize for head h, group g, query tiles rs
                (contiguous); fills ctx (+ctxT via DMA transpose for odd
                heads, whose pair rows are then complete). Each call
                allocates its OWN psum tile: a tile cached across calls
                would still receive writes after later ring allocations,
                which breaks the pool's lifetime tracking (observed as a
                nondeterministic accuracy race)."""
                exl = ex_tiles[(h // 2, g)]
                ctx_t = _ctx_for(g)
                hsl = bass.ds(64 * h, 64)
                av = qkps.tile([128, 4, 66], F32, tag="av", name=f"av{h}", bufs=2)
                for r in rs:
                    qi = 4 * g + r
                    for kj in range(qi + 1):
                        nc.tensor.matmul(
                            av[:, r, 0:65],
                            exl[kj][:, h % 2, bass.ds(r * 128, 128)],
                            v_sb[:, kj, bass.ds(65 * h, 65)],
                            start=(kj == 0),
                            stop=(kj == qi),
                        )
                r0, nr = rs[0], len(rs)
                rd = rdp.tile([128, 4], F32, tag="rd")
                nc.vector.reciprocal(rd[:, 0:nr], av[:, r0 : r0 + nr, 64:65])
                for i, r in enumerate(rs):
                    nc.vector.tensor_scalar_mul(
                        ctx_t[:, r, hsl], av[:, r, 0:64], rd[:, i : i + 1]
                    )
                if h % 2 == 1:  # heads (h-1, h) pair complete -> transpose
                    half = h // 2
                    for r in rs:
                        tt = 4 * g + r
                        nc.sync.dma_start(
                            ctxT[half][:, tt, :],
                            ctx_t[:, r, bass.ds(128 * half, 128)],
                            transpose=True,
                        )

            def emit_out_pair(g, p):
                """Output projection + 512KB DMA for tiles (2p, 2p+1) of
                group g."""
                og = outp.tile([128, 2, D], BF16, tag="og", name="og")
                for rr in range(2):
                    r = 2 * p + rr
                    tt = g * 4 + r
                    for half in range(2):
                        po = qkps.tile([128, 512], F32, tag="po", name="ps_o", bufs=2)
                        for i in range(2):
                            nc.tensor.matmul(
                                po[:],
                                ctxT[i][:, tt, :],
                                wp_sb[:, i, bass.ds(half * 512, 512)],
                                start=(i == 0),
                                stop=(i == 1),
                            )
                        nc.vector.tensor_copy(
                            og[:, rr, bass.ds(half * 512, 512)], po[:]
                        )
                eng = (nc.gpsimd, nc.sync)[(2 * g + p) % 2]
                eng.dma_start(
                    out[:, bass.ds((g * 4 + 2 * p) * D, 2 * D)], og[:]
                )
                if p == 1:
                    _ctx_cache.pop(g, None)

            def emit_drain_av(g, r):
                """Drain stage 1 for the LAST group, head 3, query tile r:
                AV chain -> norm -> transpose. Emittable once pair-1
                exps cover kj <= 4g+r; the lagged dr_po pipeline hides
                the transpose latency for all but the last tile."""
                h = HC - 1
                exl = ex_tiles[(h // 2, g)]
                ctx_t = _ctx_for(g)
                hsl = bass.ds(64 * h, 64)
                qi = 4 * g + r
                av = qkps.tile([128, 4, 66], F32, tag="av", name="av_dr", bufs=2)
                for kj in range(qi + 1):
                    nc.tensor.matmul(
                        av[:, 0, 0:65],
                        exl[kj][:, h % 2, bass.ds(r * 128, 128)],
                        v_sb[:, kj, bass.ds(65 * h, 65)],
                        start=(kj == 0),
                        stop=(kj == qi),
                    )
                rd = rdp.tile([128, 4], F32, tag="rd", name="rd_dr")
                nc.vector.reciprocal(rd[:, 0:1], av[:, 0, 64:65])
                nc.vector.tensor_scalar_mul(
                    ctx_t[:, r, hsl], av[:, 0, 0:64], rd[:, 0:1]
                )
                nc.sync.dma_start(
                    ctxT[1][:, 4 * g + r, :],
                    ctx_t[:, r, bass.ds(128, 128)],
                    transpose=True,
                )

            _dr_od = {}

            def emit_drain_po(g, r):
                """Drain stage 2: out-proj for tile r, lagged one tile
                behind its transpose. Output accumulates into a 2-tile
                [128,2,D] buffer DMA'd once per PAIR with 4KB-contiguous
                descriptors: the kernel tail is limited by the DMA
                queues' ~23ns/descriptor feed rate, not bytes, so
                halving descriptor count directly shortens the drain."""
                tt = 4 * g + r
                if r % 2 == 0:
                    # "og" tag ring: no other og allocation is emitted
                    # between this one and its pair-completing DMA below
                    _dr_od["t"] = outp.tile([128, 2, D], BF16, tag="og", name="od")
                od = _dr_od["t"]
                for half in range(2):
                    po = qkps.tile([128, 512], F32, tag="po", name="ps_o", bufs=2)
                    for i in range(2):
                        nc.tensor.matmul(
                            po[:],
                            ctxT[i][:, tt, :],
                            wp_sb[:, i, bass.ds(half * 512, 512)],
                            start=(i == 0),
                            stop=(i == 1),
                        )
                    osl = bass.ds(half * 512, 512)
                    # split drain casts: ScalarE is free after exps
                    if half == 1:
                        nc.scalar.copy(od[:, r % 2, osl], po[:])
                    else:
                        nc.vector.tensor_copy(od[:, r % 2, osl], po[:])
                if r % 2 == 1:
                    eng = (nc.gpsimd, nc.sync)[(r // 2) % 2]
                    eng.dma_start(
                        out[:, bass.ds((tt - 1) * D, 2 * D)], od[:]
                    )
                if r == 3:
                    _ctx_cache.pop(g, None)

            # ---- schedule ---------------------------------------------
            # Zipper: 2-kj score chunks (0.45us PE feeding 1.9us of Exp)
            # alternate with ~2us filler units so the Exp stream runs
            # continuously without the in-order PE stream ever blocking
            # on a qk psum buffer. AV r-chains are emitted only after the
            # score chunks covering their kj range (PE is in-order: an AV
            # matmul emitted before its exp's score matmul would deadlock).
            s, pq, pv = emit_scores, emit_proj_qk, emit_proj_v
            av, op = emit_av, emit_out_pair
            dr_av, dr_po = emit_drain_av, emit_drain_po
            # round 0 (PE-rich: projections dominate; exp stream has slack)
            pq(0, halves=(0,))
            s(0, 0, (0, 1))
            pq(0, halves=(1,), which="q")
            s(0, 0, (2, 3))
            pq(0, halves=(1,), which="k")
            s(1, 0, (0, 1))
            pv(0, (0, 1))
            s(1, 0, (2, 3))
            pv(0, (2, 3))
            av(0, 0); av(1, 0)
            pq(1, halves=(0,))
            av(2, 0); av(3, 0)
            pq(1, halves=(1,))
            # round 1
            s(0, 1, (0, 1)); pv(1, (0, 1))
            s(0, 1, (2, 3)); pv(1, (2, 3))
            s(0, 1, (4, 5)); pq(2, halves=(0,), which="q")
            s(0, 1, (6, 7)); pq(2, halves=(0,), which="k")
            s(1, 1, (0, 1)); pq(2, halves=(1,), which="q")
            s(1, 1, (2, 3)); pq(2, halves=(1,), which="k")
            s(1, 1, (4, 5)); av(0, 1, (0, 1)); op(0, 0)
            s(1, 1, (6, 7)); av(0, 1, (2, 3)); av(1, 1, (0, 1))
            av(1, 1, (2, 3)); op(0, 1); av(2, 1); av(3, 1)
            # round 2 (scalar tightens: ~1.5us filler per 2-kj chunk)
            s(0, 2, (0, 1)); pv(2, (0, 1))
            s(0, 2, (2, 3)); pv(2, (2, 3))
            s(0, 2, (4, 5)); pq(3, halves=(0,), which="q")
            s(0, 2, (6, 7)); pq(3, halves=(0,), which="k")
            s(0, 2, (8, 9)); pq(3, halves=(1,), which="q")
            s(0, 2, (10, 11)); pq(3, halves=(1,), which="k")
            s(1, 2, (0, 1)); op(1, 0)
            s(1, 2, (2, 3)); s(0, 3, (0, 1)); op(1, 1)
            s(1, 2, (4, 5)); s(0, 3, (2, 3)); av(0, 2, (0, 1))
            s(1, 2, (6, 7)); av(0, 2, (2, 3))
            s(1, 2, (8, 9)); av(1, 2, (0, 1))
            s(1, 2, (10, 11)); av(1, 2, (2, 3))
            # round 3: round-2 AV leftovers lead so their ctxT transposes
            # get ~6us of lead before op(2,*) consumes them; the first 4
            # pair-0 kjs were pulled into round 2's scalar slack above
            s(0, 3, (4, 5)); av(2, 2, (0, 1))
            s(0, 3, (6, 7)); av(2, 2, (2, 3))
            s(0, 3, (8, 9)); av(3, 2, (0, 1))
            s(0, 3, (10, 11)); av(3, 2, (2, 3))
            s(0, 3, (12,)); pv(3, (0, 1))
            s(0, 3, (13,)); pv(3, (2, 3))
            s(0, 3, (14,)); op(2, 0)
            s(0, 3, (15,)); av(0, 3, (0, 1))
            s(1, 3, (0, 1)); av(0, 3, (2, 3))
            s(1, 3, (2, 3)); av(1, 3, (0, 1))
            s(1, 3, (4, 5)); av(1, 3, (2, 3))
            s(1, 3, (6, 7)); op(2, 1)
            s(1, 3, (8, 9))
            s(1, 3, (10, 11))
            s(1, 3, (12,)); av(2, 3, (0,))
            s(1, 3, (13,)); av(2, 3, (1,)); dr_av(3, 0)
            s(1, 3, (14,)); av(2, 3, (2,)); dr_av(3, 1); dr_po(3, 0)
            s(1, 3, (15,)); dr_av(3, 2); av(2, 3, (3,)); dr_po(3, 1)
            dr_av(3, 3); dr_po(3, 2); dr_po(3, 3)
            # av-tag psum ring (2 slots) allocation order in this tail:
            # av23r0 A, av23r1 B, drav0 A, av23r2 B, drav1 A, drav2 B,
            # av23r3 A, drav3 B -- every allocation's slot predecessor is
            # fully emitted and consumed by then.

    return nc


_NC_CACHE = None


def _get_nc():
    global _NC_CACHE
    if _NC_CACHE is None:
        nc = _build_nc()
        nc.finalize()  # runs Bacc's pass pipeline (sync-wait splitting etc.)
        _NC_CACHE = nc
    return _NC_CACHE


def _make_in_maps(x, W_qkv, b_qkv, W_proj):
    tri = np.triu(np.ones((128, 128), dtype=np.float32)).astype(NPBF16)

    def wtile(w):  # [D, M] -> [128, KS, M] contraction-major tiles
        m = w.shape[1]
        return np.ascontiguousarray(
            w.astype(NPBF16).reshape(KS, 128, m).transpose(1, 0, 2)
        )

    def wtile2(w):  # [D, 256] -> [128, 2 halves, KS, 128] half-major
        return np.ascontiguousarray(
            w.astype(NPBF16).reshape(KS, 128, 2, 128).transpose(1, 2, 0, 3)
        )

    # xT per batch: [S, D] -> [128, NCOL, KS, 512]
    xTs = [
        np.ascontiguousarray(
            x[b]
            .astype(NPBF16)
            .reshape(NCOL, 512, KS, 128)
            .transpose(3, 0, 2, 1)
        )
        for b in range(B)
    ]

    in_maps = []
    for c in range(8):
        b = c // 4
        hs = [4 * (c % 4) + i for i in range(HC)]
        cs = np.concatenate([np.arange(64 * h, 64 * h + 64) for h in hs])
        wq_c = W_qkv[:, 0 * D :][:, cs]                      # [D, 256]
        wk_c = W_qkv[:, 1 * D :][:, cs]
        v_blk = W_qkv[:, 2 * D :][:, cs].astype(np.float32)  # [D, 256]
        wv_c = np.zeros((D, 260), dtype=np.float32)
        bv_c = np.zeros((1, 260), dtype=np.float32)
        for i in range(HC):
            wv_c[:, 65 * i : 65 * i + 64] = v_blk[:, 64 * i : 64 * i + 64]
            bv_c[0, 65 * i : 65 * i + 64] = b_qkv[2 * D :][cs][64 * i : 64 * i + 64]
            bv_c[0, 65 * i + 64] = 1.0
        in_maps.append(
            {
                "xT": xTs[b],
                "wq": wtile2(wq_c),
                "wk": wtile2(wk_c),
                "wv": wtile(wv_c),
                "bq": np.ascontiguousarray(
                    b_qkv[0 * D :][cs].astype(np.float32).reshape(2, 128).T
                ),
                "bk": np.ascontiguousarray(
                    b_qkv[1 * D :][cs].astype(np.float32).reshape(2, 128).T
                ),
                "bv": bv_c.astype(NPBF16),
                "wp": np.ascontiguousarray(
                    W_proj[cs, :].astype(NPBF16).reshape(2, 128, D).transpose(1, 0, 2)
                ),
                "tri": tri,
            }
        )
    return in_maps


def kernel(x, W_qkv, b_qkv, W_proj, b_proj, **run_kwargs):
    x = np.asarray(x, dtype=np.float32)
    W_qkv = np.asarray(W_qkv, dtype=np.float32)
    b_qkv = np.asarray(b_qkv, dtype=np.float32)
    W_proj = np.asarray(W_proj, dtype=np.float32)
    b_proj = np.asarray(b_proj, dtype=np.float32)

    nc = _get_nc()
    in_maps = _make_in_maps(x, W_qkv, b_qkv, W_proj)
    res = run_bass_kernel_spmd(nc, in_maps, core_ids=list(range(8)), **run_kwargs)

    out = np.zeros((B, S, D), dtype=np.float32)
    for c in range(8):
        b = c // 4
        # o[p, tt*D + d] = out[tt*128 + p, d]
        o = res.results[c]["o"].astype(np.float32).reshape(128, QT, D)
        out[b] += o.transpose(1, 0, 2).reshape(S, D)
    out += b_proj[None, None, :]
    kernel.last_result = res
    return out
